# revision 56
# baseline (speedup 1.0000x reference)
"""DGI (2-layer GCN encoder x2 + bilinear discriminator) on 8 Trainium2 cores.

Strategy
--------
Both encodes share the graph, so they are fused into one 128-wide feature
matrix ([x-encode 64 | cfeat-encode 64]).  The symmetric GCN normalization is
factored into row scalings:  A_hat @ H = diag(dinv) @ A01 @ (diag(dinv) @ H),
where A01 is the 0/1 adjacency (incl. self loops).  The SpMM against A01 is
computed per destination block of 128 nodes as a sum of one-hot matmuls
accumulating in PSUM; the aggregation is produced TRANSPOSED ([feat x dst],
lhsT = gathered messages, rhs = dst-one-hots) so the second-layer transform
needs no per-block transposes, and the dst-side dinv scaling folds into the
next consumer (relu commutes with a positive diagonal scale: layer-2's table
gets dinv^2, the discriminator applies dinv per-partition).

Sharding: nodes are split into 8 contiguous ranges (12500/core, padded to
12544).  Each core computes its rows of the gather table (dense matmul),
AllGathers the full bf16 table in TWO halves (each core's blocks 0-48 then
49-97), then processes edges whose dst lands in its range.  Edges are
pre-sorted by (dst window, src chunk); the 4 equal src chunks of 25088 rows
align with the AllGather halves, so the first half's gathers overlap the
second half's collective, and map 1:1 onto the 4 SWDGE queues (disjoint Q7
core pairs -> concurrent descriptor generation).

Discriminator reduces to  sc = sigmoid(dinv*mean(agg1)) * dinv*(agg @ rowsum(Wd)) + bd,
computed per dst block as one [128x4] matmul off the transposed aggregation.
"""

import numpy as np
import ml_dtypes

import concourse.bass as bass
import concourse.bacc as bacc
import concourse.mybir as mybir
import concourse.tile as tile
from concourse import bass_utils
from concourse.library_config import mlp

BF16 = ml_dtypes.bfloat16

N = 100000
E = 1600000
IN_D = 128
HID = 64
OUT_D = 64
C = 8                 # cores
S = N // C            # 12500 nodes per core
B = 98                # dst blocks of 128 per core (98*128 = 12544)
SP = B * 128          # padded shard rows
# table split into 4 sections of 25/24/25/24 blocks; each section is its own
# AllGather + gather chunk + SWDGE queue, so gathers start as soon as their
# section's collective lands
SEC = [0, 3200, 6272, 9472, 12544]          # local row boundaries
SECG = [0, 25600, 50176, 75776, 100352]     # global (post-gather) boundaries
CB = SECG
NCH = 4
G = 7                 # dst blocks per window
NW = B // G           # 14 windows
P = 128


def _preprocess(edge_index, mode):
    """Sort/pad edges into the per-core streamed tile layout.

    mode "half": table layout = [half, core] (2 collectives; chunk = half x
    core-group).  mode "sec4": [section, core] (4 collectives; chunk =
    section).  Returns per-core idx/dloc arrays plus the tile schedule.
    """
    ei = np.asarray(edge_index).astype(np.int64)
    src = ei[0]
    dst = ei[1]
    # degree includes the self loop; the self-loop message itself is not
    # gathered -- it is added on-device from the core's own table rows.
    deg = (np.bincount(dst, minlength=N) + 1).astype(np.float32)
    dinv = (1.0 / np.sqrt(deg)).astype(np.float32)

    core = dst // S
    blk = (dst % S) // P
    dloc = (dst % S) % P
    srccore = src // S
    sl = src % S
    if mode == "half":
        HB = 6272
        half = (sl >= HB).astype(np.int64)
        prow = half * (HB * C) + srccore * HB + (sl - half * HB)
        cb = np.asarray([0, 25088, 50176, 75264, 100352])
    else:
        sec = np.searchsorted(np.asarray(SEC), sl, side="right") - 1
        seclen = np.asarray([SEC[i + 1] - SEC[i] for i in range(4)])
        prow = (np.asarray(SECG)[sec] + srccore * seclen[sec]
                + (sl - np.asarray(SEC)[sec]))
        cb = np.asarray(SECG)
    chk = np.searchsorted(cb, prow, side="right") - 1
    sloc = (prow - cb[chk]).astype(np.int32)
    assert sloc.max() < 25600 and sloc.min() >= 0

    nseg = B * NCH
    segkey = (core * B + blk) * NCH + chk
    # secondary sort by source row: the SDMA drain reads each run's 256B
    # rows in ascending HBM order (better row-buffer locality)
    order = np.lexsort((sloc, segkey))
    segkey_s = segkey[order]
    sloc_s = sloc[order]
    dloc_s = dloc[order]

    cnt = np.bincount(segkey, minlength=C * nseg).reshape(C, B, NCH)

    # stream order: window-major, chunk-major inside a window.  Each core
    # packs its blocks' runs back-to-back inside the (window, chunk) segment
    # (per-core offsets -- the masked per-core dloc columns define block
    # membership per slot), so only the segment length is a cross-core max.
    # The matmul list covers the union of (tile, block) pairs across cores;
    # a core with no messages for a pair contributes an all-255 column.
    seg_base_core = np.zeros((C, B * NCH), np.int64)
    slotblk_core = [[] for _ in range(C)]
    schedule = []
    t = 0              # stream tiles
    for w in range(NW):
        wsched = {"tile0": t, "chunks": [], "mms": []}
        blocks = list(range(w * G, (w + 1) * G))
        lo = {}        # (b) -> min slot over cores, per chunk below
        for c in range(NCH):
            c0 = t
            seg0 = t * P
            pair_lo = {b: None for b in blocks}
            pair_hi = {b: None for b in blocks}
            seglen = 0
            for r in range(C):
                off = seg0
                for b in blocks:
                    n = int(cnt[r, b, c])
                    seg_base_core[r, b * NCH + c] = off
                    if n:
                        if pair_lo[b] is None or off < pair_lo[b]:
                            pair_lo[b] = off
                        if pair_hi[b] is None or off + n > pair_hi[b]:
                            pair_hi[b] = off + n
                        slotblk_core[r].extend([b] * n)
                    off += n
                seglen = max(seglen, off - seg0)
                slotblk_core[r].extend(
                    [-1] * (-(-seglen // P) * P - (off - seg0)))
            nt = -(-seglen // P)
            # pad every core's slot map to the segment tile boundary
            for r in range(C):
                need = (t + nt) * P - len(slotblk_core[r])
                slotblk_core[r].extend([-1] * need)
            t += nt
            wsched["chunks"].append((c0 - wsched["tile0"], t - c0))
            for b in blocks:
                if pair_lo[b] is not None:
                    lo[(b, c)] = (pair_lo[b], pair_hi[b])
        wsched["ntiles"] = t - wsched["tile0"]
        # matmul list: block-major (contiguous PSUM accumulation groups)
        for b in blocks:
            mms_b = []
            for c in range(NCH):
                if (b, c) not in lo:
                    continue
                s0, s1 = lo[(b, c)]
                for mt in range(s0 // P, (s1 - 1) // P + 1):
                    mms_b.append([mt - wsched["tile0"], b])
            for j, m in enumerate(mms_b):
                wsched["mms"].append(
                    (m[0], m[1], j == 0, j == len(mms_b) - 1))
        schedule.append(wsched)
    TOTT = t
    slotblk_core = [np.asarray(x, np.int64) for x in slotblk_core]
    for r in range(C):
        assert slotblk_core[r].size == TOTT * P, (r, slotblk_core[r].size, TOTT * P)

    idx_cores = []
    dloc_cores = []
    for r in range(C):
        msk = segkey_s // (B * NCH) == r
        key_r = segkey_s[msk] - r * nseg
        sl_r = sloc_s[msk]
        dl_r = dloc_s[msk]
        # rank of each message within its segment
        changes = np.r_[0, np.flatnonzero(np.diff(key_r)) + 1]
        seg_start_of_msg = np.repeat(changes, np.diff(np.r_[changes, key_r.size]))
        rank = np.arange(key_r.size) - seg_start_of_msg
        pos = seg_base_core[r][key_r] + rank

        SRC = np.zeros(TOTT * P, np.int16)
        DLC = np.full(TOTT * P, 255, np.int16)
        SRC[pos] = sl_r.astype(np.int16)
        DLC[pos] = dl_r.astype(np.int16)

        # idx packing for dma_gather: call-local index i -> [i%16, i//16],
        # replicated across the 8 groups of 16 partitions.  Calls are the
        # (window, chunk) segments; each is tile-aligned so packing the whole
        # stream at once keeps every call's columns self-contained.
        a = SRC.reshape(-1, 16).T                      # [16, TOTT*8]
        idx_cores.append(np.tile(a, (8, 1)).copy())    # [128, TOTT*8]
        # per-matmul dloc columns: slots of other blocks masked to 255
        DLCt = DLC.reshape(TOTT, P)
        SBt = slotblk_core[r].reshape(TOTT, P)
        cols = []
        for ws in schedule:
            for mt_l, b, _s, _e in ws["mms"]:
                mt = ws["tile0"] + mt_l
                cols.append(np.where(SBt[mt] == b, DLCt[mt], 255))
        dloc_cores.append(
            np.ascontiguousarray(np.stack(cols, axis=1)).astype(BF16)
        )                                              # [128, TOTC]

    TOTC = sum(len(ws["mms"]) for ws in schedule)
    return dict(
        dinv=dinv,
        schedule=schedule,
        TOTT=TOTT,
        TOTC=TOTC,
        cb=[int(x) for x in cb],
        idx_cores=idx_cores,
        dloc_cores=dloc_cores,
    )


def _build(pp1, pp2, with_b1, with_b2):
    """Build the 8-core SPMD bass program."""
    assert not with_b1 and not with_b2, "biases are zero in this problem"
    WTmax = max(ws["ntiles"] for p in (pp1, pp2) for ws in p["schedule"])
    WCmax = max(len(ws["mms"]) for p in (pp1, pp2) for ws in p["schedule"])

    nc = bacc.Bacc("TRN2", target_bir_lowering=False, debug=False, num_devices=C,
                   num_swdge_queues=4, dynamic_dma_scratch_size=32768)
    f32 = mybir.dt.float32
    bf16 = mybir.dt.bfloat16
    i16 = mybir.dt.int16

    t_xs = nc.dram_tensor("xs", [SP, P], bf16, kind="ExternalInput")
    t_cs = nc.dram_tensor("cs", [SP, P], bf16, kind="ExternalInput")
    t_w1 = nc.dram_tensor("w1", [P, HID], bf16, kind="ExternalInput")
    t_w2d = nc.dram_tensor("w2d", [P, P], bf16, kind="ExternalInput")
    t_dcols = nc.dram_tensor("dcols", [P, 4], bf16, kind="ExternalInput")
    t_iota = nc.dram_tensor("iota", [P, P], bf16, kind="ExternalInput")
    t_dinv = nc.dram_tensor("dinvc", [P, B], f32, kind="ExternalInput")
    t_dinv2 = nc.dram_tensor("dinv2c", [P, B], f32, kind="ExternalInput")
    t_idx1 = nc.dram_tensor("idx1", [P, pp1["TOTT"] * 8], i16,
                            kind="ExternalInput")
    t_dloc1 = nc.dram_tensor("dloc1", [P, pp1["TOTC"]], bf16,
                             kind="ExternalInput")
    t_idx2 = nc.dram_tensor("idx2", [P, pp2["TOTT"] * 8], i16,
                            kind="ExternalInput")
    t_dloc2 = nc.dram_tensor("dloc2", [P, pp2["TOTC"]], bf16,
                             kind="ExternalInput")
    t_out = nc.dram_tensor("out", [2, B, P], f32, kind="ExternalOutput")

    # separate dram tensors per table section (both the local staging shard
    # and the gathered copy): tile tracks DRAM deps per tensor, so each
    # section's collective fires as soon as its own rows are staged, and each
    # chunk's gathers wait only on their own collective
    t1_sh = nc.dram_tensor("t1sh", [SP, P], bf16, kind="Internal")
    t2_sh = nc.dram_tensor("t2sh", [SP, P], bf16, kind="Internal")
    t1_ss = [nc.dram_tensor(f"t1s{s}", [SECG[s + 1] - SECG[s], P], bf16,
                            kind="Internal", addr_space="Shared")
             for s in range(4)]
    t2_ss = [nc.dram_tensor(f"t2s{s}", [SECG[s + 1] - SECG[s], P], bf16,
                            kind="Internal", addr_space="Shared")
             for s in range(4)]

    Copy = mybir.ActivationFunctionType.Copy
    Relu = mybir.ActivationFunctionType.Relu
    Sigmoid = mybir.ActivationFunctionType.Sigmoid

    with tile.TileContext(nc) as tc:
        nc.gpsimd.load_library(mlp)
        with (
            tc.tile_pool(name="const", bufs=1) as constp,
            tc.tile_pool(name="hbuf", bufs=1) as hbufp,
            tc.tile_pool(name="io", bufs=3) as iop,
            tc.tile_pool(name="idx", bufs=5) as idxp,
            tc.tile_pool(name="msgs", bufs=2) as msgp,
            tc.tile_pool(name="oh", bufs=1) as ohp,
            tc.tile_pool(name="psA", bufs=2, space="PSUM") as psA,
            tc.tile_pool(name="psW", bufs=2, space="PSUM") as psW,
            tc.tile_pool(name="psD", bufs=1, space="PSUM") as psD,
            tc.tile_pool(name="small", bufs=4) as smallp,
        ):
            # ---- constants ----
            w1_sb = constp.tile([P, HID], bf16)
            nc.sync.dma_start(w1_sb[:], t_w1.ap())
            w2d_sb = constp.tile([P, P], bf16)
            nc.sync.dma_start(w2d_sb[:], t_w2d.ap())
            dcols_sb = constp.tile([P, 4], bf16)
            nc.sync.dma_start(dcols_sb[:], t_dcols.ap())
            iota_sb = constp.tile([P, P], bf16)
            nc.sync.dma_start(iota_sb[:], t_iota.ap())
            dinv_sb = constp.tile([P, B], f32)
            nc.sync.dma_start(dinv_sb[:], t_dinv.ap())
            dinv2_sb = constp.tile([P, B], f32)
            nc.sync.dma_start(dinv2_sb[:], t_dinv2.ap())
            ident_sb = constp.tile([P, P], f32)
            from concourse.masks import make_identity
            make_identity(nc, ident_sb[:])
            ident_bf = constp.tile([P, P], bf16)
            nc.vector.tensor_copy(ident_bf[:], ident_sb[:])

            hT_buf = hbufp.tile([P, B * P], bf16)    # layer-1 relu'd agg, [feat x node]
            own_buf = hbufp.tile([P, B * P], bf16)   # this core's table rows [node x feat]

            def ag(sh, out_t, r0, r1):
                nc.gpsimd.collective_compute(
                    "AllGather", mybir.AluOpType.bypass,
                    replica_groups=[list(range(C))],
                    ins=[sh.ap()[r0:r1, :]], outs=[out_t.ap()],
                )

            def write_sh(sh, b0, b1):
                nc.sync.dma_start(
                    sh.ap()[b0 * P:b1 * P, :]
                        .rearrange("(b p) f -> p b f", p=P),
                    own_buf[:, b0 * P:b1 * P]
                        .rearrange("p (b f) -> p b f", f=P))

            # ---- phase A: T1 = dinv * [x@W1 | c@W1]  (bf16 table) ----
            # the 4 section AllGathers launch as soon as their rows are done
            GA = 7
            for g0 in range(0, B, GA):
                ng = min(GA, B - g0)
                xt = iop.tile([P, GA * P], bf16, tag="xt")
                nc.sync.dma_start(xt[:, :ng * P],
                                  t_xs.ap()[g0 * P:(g0 + ng) * P, :],
                                  transpose=True)
                ct = iop.tile([P, GA * P], bf16, tag="ct")
                nc.sync.dma_start(ct[:, :ng * P],
                                  t_cs.ap()[g0 * P:(g0 + ng) * P, :],
                                  transpose=True)
                psg = psW.tile([P, G * P], f32, tag="psw")
                for j in range(ng):
                    nc.tensor.matmul(psg[:, j * P:j * P + HID],
                                     xt[:, j * P:(j + 1) * P],
                                     w1_sb[:], start=True, stop=True)
                    nc.tensor.matmul(psg[:, j * P + HID:(j + 1) * P],
                                     ct[:, j * P:(j + 1) * P],
                                     w1_sb[:], start=True, stop=True)
                # one batched scale per group (per-block dinv columns)
                nc.vector.tensor_tensor(
                    out=own_buf[:, g0 * P:(g0 + ng) * P]
                        .rearrange("p (b f) -> p b f", f=P),
                    in0=psg[:, :ng * P].rearrange("p (b f) -> p b f", f=P),
                    in1=dinv_sb[:, g0:g0 + ng]
                        .rearrange("p (b q) -> p b q", q=1)
                        .to_broadcast([P, ng, P]),
                    op=mybir.AluOpType.mult)
                write_sh(t1_sh, g0, g0 + ng)
                for s in range(4):
                    if (g0 + ng) * P >= SEC[s + 1] > g0 * P:
                        ag(t1_sh, t1_ss[s], SEC[s], SEC[s + 1])

            # num_idxs register per distinct size: avoids a MOVE (and its
            # WAR stall on the shared scratch register) before every gather
            nidx_regs = {}

            def nidx_reg(n):
                if n not in nidx_regs:
                    nidx_regs[n] = nc.gpsimd.to_reg(n)
                return nidx_regs[n]

            sc1_st = constp.tile([P, B], f32)
            sc2_st = constp.tile([P, B], f32)

            # ---- SpMM pass (shared for both layers) ----
            def spmm(pp, t_idx, t_dloc, tables, layer, pre, agb):
                schedule = pp["schedule"]
                cbs = pp["cb"]
                k0s = [0]
                for ws in schedule:
                    k0s.append(k0s[-1] + len(ws["mms"]))
                tiles = {}

                def load(w):
                    ws = schedule[w]
                    wt = ws["ntiles"]
                    t0 = ws["tile0"]
                    wc = len(ws["mms"])
                    idxw = idxp.tile([P, WTmax * 8], i16, tag="idxw")
                    nc.scalar.dma_start(idxw[:, :wt * 8],
                                        t_idx.ap()[:, t0 * 8:(t0 + wt) * 8])
                    dlocw = idxp.tile([P, WCmax], bf16, tag="dlocw")
                    nc.scalar.dma_start(dlocw[:, :wc],
                                        t_dloc.ap()[:, k0s[w]:k0s[w] + wc])
                    msgs = msgp.tile([P, WTmax * P], bf16, tag="msgs")
                    tiles[w] = (idxw, dlocw, msgs)

                def gather(w, chunks):
                    ws = schedule[w]
                    idxw, _, msgs = tiles[w]
                    # one gather per (chunk == SWDGE queue): the 4 queues run
                    # on disjoint Q7 core pairs, concurrently, and the equal
                    # chunk sizes keep them balanced
                    for c in chunks:
                        coff, cnt_t = ws["chunks"][c]
                        tbl, r0 = tables[c]
                        rows = cbs[c + 1] - cbs[c]
                        for s0 in range(0, cnt_t, 44):
                            st = min(44, cnt_t - s0)
                            o = coff + s0
                            nidx = st * P
                            nc.gpsimd.dma_gather(
                                msgs[:, o * P:(o + st) * P].rearrange(
                                    "p (t d) -> p t d", d=P),
                                tbl.ap()[r0:r0 + rows, :],
                                idxw[:, o * 8:(o + st) * 8],
                                nidx, nidx_reg(nidx), P, single_packet=False,
                                queue_num=(c + w) % 4,
                            )

                def compute(w):
                    ws = schedule[w]
                    wc = len(ws["mms"])
                    _, dlocw, msgs = tiles[w]
                    ohg = ohp.tile([P, WCmax * P], bf16, tag="ohg")
                    # build the one-hots in pieces so the first matmuls can
                    # start while DVE still builds the tail
                    ohc = -(-wc // 3)
                    for j0 in range(0, wc, ohc):
                        jn = min(ohc, wc - j0)
                        nc.vector.tensor_tensor(
                            out=ohg[:, j0 * P:(j0 + jn) * P]
                                .rearrange("p (t d) -> p t d", d=P),
                            in0=dlocw[:, j0:j0 + jn].to_broadcast([P, jn, P]),
                            in1=iota_sb[:].rearrange("p (a d) -> p a d", a=1)
                                .to_broadcast([P, jn, P]),
                            op=mybir.AluOpType.is_equal)
                    # transposed aggregation: psw[feat, dst] += msgs^T @ onehot
                    psw = psW.tile([P, G * P], f32, tag="psw")
                    for k, (mt_l, b, st_f, sp_f) in enumerate(ws["mms"]):
                        bw = b - w * G
                        if st_f:
                            # self-loop: psum[:, d] += own_buf[d, :]^T
                            nc.tensor.matmul(
                                psw[:, bw * P:(bw + 1) * P],
                                own_buf[:, b * P:(b + 1) * P], ident_bf[:],
                                start=True, stop=False)
                        nc.tensor.matmul(
                            psw[:, bw * P:(bw + 1) * P],
                            msgs[:, mt_l * P:(mt_l + 1) * P],
                            ohg[:, k * P:(k + 1) * P],
                            start=False, stop=sp_f)
                    if layer == 1:
                        for bw in range(G):
                            gb = w * G + bw
                            # h~ = relu(agg); dst dinv deferred (relu commutes
                            # with the positive diagonal scale)
                            nc.scalar.activation(
                                hT_buf[:, gb * P:(gb + 1) * P],
                                psw[:, bw * P:(bw + 1) * P], Relu)
                            # phase C pipelined: T2 = dinv^2 * (h~ @ W2d)
                            ps = psA.tile([P, P], f32, tag="psd")
                            nc.tensor.matmul(ps[:],
                                             hT_buf[:, gb * P:(gb + 1) * P],
                                             w2d_sb[:], start=True, stop=True)
                            nc.scalar.activation(own_buf[:, gb * P:(gb + 1) * P],
                                                 ps[:], Copy,
                                                 scale=dinv2_sb[:, gb:gb + 1])
                        write_sh(t2_sh, w * G, (w + 1) * G)
                        # T2 section collectives launch mid-stream, masked
                        # by the in-flight gathers
                        for s in range(3):
                            if (w + 1) * G * P >= SEC[s + 1] > w * G * P:
                                ag(t2_sh, t2_ss[s], SEC[s], SEC[s + 1])
                    else:
                        # discriminator, inline: per block one [128x4] matmul
                        # off the (unscaled) aggregation copy
                        y2w = smallp.tile([P, G * P], bf16, tag="y2w")
                        psd = psD.tile([P, 4 * G], f32, tag="psd2")
                        for bw in range(G):
                            nc.scalar.activation(
                                y2w[:, bw * P:(bw + 1) * P],
                                psw[:, bw * P:(bw + 1) * P], Copy)
                            nc.tensor.matmul(
                                psd[:, bw * 4:(bw + 1) * 4],
                                y2w[:, bw * P:(bw + 1) * P], dcols_sb[:],
                                start=True, stop=True)
                        # scale all rows by dst dinv, then
                        # sc_j = (w . y2) * sigmoid(dinv * mean(h1-part))
                        sd = smallp.tile([P, 4 * G], f32, tag="sd")
                        sdv = sd[:].rearrange("p (b q) -> p b q", q=4)
                        nc.vector.tensor_tensor(
                            out=sdv,
                            in0=psd[:].rearrange("p (b q) -> p b q", q=4),
                            in1=dinv_sb[:, w * G:(w + 1) * G]
                                .rearrange("p (b q) -> p b q", q=1)
                                .to_broadcast([P, G, 4]),
                            op=mybir.AluOpType.mult)
                        ccol = smallp.tile([P, G], f32, tag="ccol")
                        ccol3 = ccol[:].rearrange("p (b q) -> p b q", q=1)
                        nc.scalar.activation(ccol3, sdv[:, :, 0:1], Sigmoid)
                        nc.vector.tensor_tensor(
                            out=sc1_st[:, w * G:(w + 1) * G]
                                .rearrange("p (b q) -> p b q", q=1),
                            in0=sdv[:, :, 1:2],
                            in1=ccol3, op=mybir.AluOpType.mult)
                        nc.vector.tensor_tensor(
                            out=sc2_st[:, w * G:(w + 1) * G]
                                .rearrange("p (b q) -> p b q", q=1),
                            in0=sdv[:, :, 2:3],
                            in1=ccol3, op=mybir.AluOpType.mult)
                        # output per table half: transpose sc -> [blk, P]
                        # and store, so the tail only drains the last half
                        if w in (6, NW - 1):
                            b0, b1 = (0, 49) if w == 6 else (49, B)
                            for j, st in enumerate((sc1_st, sc2_st)):
                                pso = psD.tile([49, P], f32, tag="pstr")
                                nc.tensor.transpose(pso[:], st[:, b0:b1],
                                                    ident_sb[:])
                                so = smallp.tile([49, P], f32, tag="so")
                                nc.scalar.activation(so[:], pso[:], Copy)
                                nc.sync.dma_start(t_out.ap()[j][b0:b1, :],
                                                  so[:])

                # prelude: the first two windows' already-gatherable chunks
                # are emitted BEFORE the last collective, so the Pool engine
                # streams useful gathers while the collective's data-wait +
                # CC execution run
                rest = [c for c in range(4) if c not in pre]
                load(0)
                gather(0, pre)
                load(1)
                gather(1, pre)
                agb()
                gather(0, rest)
                gather(1, rest)
                compute(0)
                compute(1)
                for w in range(2, NW):
                    load(w)
                    gather(w, [0, 1, 2, 3])
                    compute(w)

            spmm(pp1, t_idx1, t_dloc1,
                 [(t1_ss[0], 0), (t1_ss[1], 0), (t1_ss[2], 0), (t1_ss[3], 0)],
                 layer=1, pre=[0, 1, 2, 3], agb=lambda: None)
            spmm(pp2, t_idx2, t_dloc2,
                 [(t2_ss[0], 0), (t2_ss[1], 0), (t2_ss[2], 0), (t2_ss[3], 0)],
                 layer=2, pre=[0, 1, 2],
                 agb=lambda: ag(t2_sh, t2_ss[3], SEC[3], SP))

    nc.compile()
    return nc


_CACHE = {}


def _run(inputs, trace=False):
    x = np.asarray(inputs["x"], np.float32)
    cfeat = np.asarray(inputs["cfeat"], np.float32)
    edge_index = inputs["edge_index"]
    W1 = np.asarray(inputs["W1"], np.float32)
    b1 = np.asarray(inputs["b1"], np.float32)
    W2 = np.asarray(inputs["W2"], np.float32)
    b2 = np.asarray(inputs["b2"], np.float32)
    Wd = np.asarray(inputs["Wd"], np.float32)
    bd = np.asarray(inputs["bd"], np.float32)

    pp1 = _preprocess(edge_index, "sec4")
    pp2 = pp1
    with_b1 = bool(np.any(b1 != 0))
    with_b2 = bool(np.any(b2 != 0))

    key = ("nc", with_b1, with_b2, int(np.asarray(edge_index)[0, 0]),
           pp1["TOTT"], pp2["TOTT"])
    if key not in _CACHE:
        _CACHE.clear()
        _CACHE[key] = _build(pp1, pp2, with_b1, with_b2)
    nc = _CACHE[key]

    w = Wd.sum(axis=1).astype(np.float32)          # [64]
    # disc matmul columns: [mean-input(h1) | w.h1 | w.h2 | 0]
    dcols = np.zeros((P, 4), np.float32)
    dcols[:HID, 0] = 1.0 / HID
    dcols[:HID, 1] = w
    dcols[HID:, 2] = w
    dcols = dcols.astype(BF16)
    w1b = W1.astype(BF16)
    w2d = np.zeros((P, P), np.float32)
    w2d[:HID, :HID] = W2
    w2d[HID:, HID:] = W2
    w2d = w2d.astype(BF16)
    iota = np.tile(np.arange(P, dtype=np.float32)[None, :], (P, 1)).astype(BF16)

    dinv = pp1["dinv"]
    in_maps = []
    for r in range(C):
        xs = np.zeros((SP, P), np.float32)
        xs[:S] = x[r * S:(r + 1) * S]
        cs = np.zeros((SP, P), np.float32)
        cs[:S] = cfeat[r * S:(r + 1) * S]
        dv = np.ones(SP, np.float32)
        dv[:S] = dinv[r * S:(r + 1) * S]
        dvc = np.ascontiguousarray(dv.reshape(B, P).T)
        in_maps.append(dict(
            xs=xs.astype(BF16), cs=cs.astype(BF16),
            w1=w1b, w2d=w2d, dcols=dcols, iota=iota,
            dinvc=dvc, dinv2c=dvc * dvc,
            idx1=pp1["idx_cores"][r], dloc1=pp1["dloc_cores"][r],
            idx2=pp2["idx_cores"][r], dloc2=pp2["dloc_cores"][r],
        ))

    res = bass_utils.run_bass_kernel_spmd(
        nc, in_maps, core_ids=list(range(C)), trace=trace)

    sc1 = np.empty(N, np.float32)
    sc2 = np.empty(N, np.float32)
    for r in range(C):
        o = res.results[r]["out"].reshape(2, SP)
        sc1[r * S:(r + 1) * S] = o[0, :S]
        sc2[r * S:(r + 1) * S] = o[1, :S]
    logits = np.concatenate([sc1 + bd[0], sc2 + bd[0]])[None, :].astype(np.float32)
    return logits, res


def kernel(**inputs):
    logits, _ = _run(inputs, trace=False)
    return logits


# revision 57
# speedup vs baseline: 1.0797x; 1.0797x over previous
"""DGI (2-layer GCN encoder x2 + bilinear discriminator) on 8 Trainium2 cores.

Strategy
--------
Both encodes share the graph, so they are fused into one 128-wide feature
matrix ([x-encode 64 | cfeat-encode 64]).  The symmetric GCN normalization is
factored into row scalings:  A_hat @ H = diag(dinv) @ A01 @ (diag(dinv) @ H),
where A01 is the 0/1 adjacency (incl. self loops).  The SpMM against A01 is
computed per destination block of 128 nodes as a sum of one-hot matmuls
accumulating in PSUM; the aggregation is produced TRANSPOSED ([feat x dst],
lhsT = gathered messages, rhs = dst-one-hots) so the second-layer transform
needs no per-block transposes, and the dst-side dinv scaling folds into the
next consumer (relu commutes with a positive diagonal scale: layer-2's table
gets dinv^2, the discriminator applies dinv per-partition).

Sharding: nodes are split into 8 contiguous ranges (12500/core, padded to
12544).  Each core computes its rows of the gather table (dense matmul),
AllGathers the full bf16 table in TWO halves (each core's blocks 0-48 then
49-97), then processes edges whose dst lands in its range.  Edges are
pre-sorted by (dst window, src chunk); the 4 equal src chunks of 25088 rows
align with the AllGather halves, so the first half's gathers overlap the
second half's collective, and map 1:1 onto the 4 SWDGE queues (disjoint Q7
core pairs -> concurrent descriptor generation).

Discriminator reduces to  sc = sigmoid(dinv*mean(agg1)) * dinv*(agg @ rowsum(Wd)) + bd,
computed per dst block as one [128x4] matmul off the transposed aggregation.
"""

import numpy as np
import ml_dtypes

import concourse.bass as bass
import concourse.bacc as bacc
import concourse.mybir as mybir
import concourse.tile as tile
from concourse import bass_utils
from concourse.library_config import mlp

BF16 = ml_dtypes.bfloat16

N = 100000
E = 1600000
IN_D = 128
HID = 64
OUT_D = 64
C = 8                 # cores
S = N // C            # 12500 nodes per core
B = 98                # dst blocks of 128 per core (98*128 = 12544)
SP = B * 128          # padded shard rows
# table split into 4 sections of 25/24/25/24 blocks; each section is its own
# AllGather + gather chunk + SWDGE queue, so gathers start as soon as their
# section's collective lands
SEC = [0, 3200, 6272, 9472, 12544]          # local row boundaries
SECG = [0, 25600, 50176, 75776, 100352]     # global (post-gather) boundaries
CB = SECG
NCH = 4
G = 7                 # dst blocks per window
NW = B // G           # 14 windows
P = 128


def _preprocess(edge_index, mode):
    """Sort/pad edges into the per-core streamed tile layout.

    mode "half": table layout = [half, core] (2 collectives; chunk = half x
    core-group).  mode "sec4": [section, core] (4 collectives; chunk =
    section).  Returns per-core idx/dloc arrays plus the tile schedule.
    """
    ei = np.asarray(edge_index).astype(np.int64)
    src = ei[0]
    dst = ei[1]
    # degree includes the self loop; the self-loop message itself is not
    # gathered -- it is added on-device from the core's own table rows.
    deg = (np.bincount(dst, minlength=N) + 1).astype(np.float32)
    dinv = (1.0 / np.sqrt(deg)).astype(np.float32)

    core = dst // S
    blk = (dst % S) // P
    dloc = (dst % S) % P
    srccore = src // S
    sl = src % S
    if mode == "half":
        HB = 6272
        half = (sl >= HB).astype(np.int64)
        prow = half * (HB * C) + srccore * HB + (sl - half * HB)
        cb = np.asarray([0, 25088, 50176, 75264, 100352])
    else:
        sec = np.searchsorted(np.asarray(SEC), sl, side="right") - 1
        seclen = np.asarray([SEC[i + 1] - SEC[i] for i in range(4)])
        prow = (np.asarray(SECG)[sec] + srccore * seclen[sec]
                + (sl - np.asarray(SEC)[sec]))
        cb = np.asarray(SECG)
    chk = np.searchsorted(cb, prow, side="right") - 1
    sloc = (prow - cb[chk]).astype(np.int32)
    assert sloc.max() < 25600 and sloc.min() >= 0

    nseg = B * NCH
    segkey = (core * B + blk) * NCH + chk
    # secondary sort by source row: the SDMA drain reads each run's 256B
    # rows in ascending HBM order (better row-buffer locality)
    order = np.lexsort((sloc, segkey))
    segkey_s = segkey[order]
    sloc_s = sloc[order]
    dloc_s = dloc[order]

    cnt = np.bincount(segkey, minlength=C * nseg).reshape(C, B, NCH)

    # stream order: window-major, chunk-major inside a window.  Each core
    # packs its blocks' runs back-to-back inside the (window, chunk) segment
    # (per-core offsets -- the masked per-core dloc columns define block
    # membership per slot), so only the segment length is a cross-core max.
    # The matmul list covers the union of (tile, block) pairs across cores;
    # a core with no messages for a pair contributes an all-255 column.
    seg_base_core = np.zeros((C, B * NCH), np.int64)
    slotblk_core = [[] for _ in range(C)]
    schedule = []
    t = 0              # stream tiles
    for w in range(NW):
        wsched = {"tile0": t, "chunks": [], "mms": []}
        blocks = list(range(w * G, (w + 1) * G))
        lo = {}        # (b) -> min slot over cores, per chunk below
        for c in range(NCH):
            c0 = t
            seg0 = t * P
            pair_lo = {b: None for b in blocks}
            pair_hi = {b: None for b in blocks}
            seglen = 0
            for r in range(C):
                off = seg0
                for b in blocks:
                    n = int(cnt[r, b, c])
                    seg_base_core[r, b * NCH + c] = off
                    if n:
                        if pair_lo[b] is None or off < pair_lo[b]:
                            pair_lo[b] = off
                        if pair_hi[b] is None or off + n > pair_hi[b]:
                            pair_hi[b] = off + n
                        slotblk_core[r].extend([b] * n)
                    off += n
                seglen = max(seglen, off - seg0)
                slotblk_core[r].extend(
                    [-1] * (-(-seglen // P) * P - (off - seg0)))
            nt = -(-seglen // P)
            # pad every core's slot map to the segment tile boundary
            for r in range(C):
                need = (t + nt) * P - len(slotblk_core[r])
                slotblk_core[r].extend([-1] * need)
            t += nt
            wsched["chunks"].append((c0 - wsched["tile0"], t - c0))
            for b in blocks:
                if pair_lo[b] is not None:
                    lo[(b, c)] = (pair_lo[b], pair_hi[b])
        wsched["ntiles"] = t - wsched["tile0"]
        # matmul list: block-major (contiguous PSUM accumulation groups)
        for b in blocks:
            mms_b = []
            for c in range(NCH):
                if (b, c) not in lo:
                    continue
                s0, s1 = lo[(b, c)]
                for mt in range(s0 // P, (s1 - 1) // P + 1):
                    mms_b.append([mt - wsched["tile0"], b])
            for j, m in enumerate(mms_b):
                wsched["mms"].append(
                    (m[0], m[1], j == 0, j == len(mms_b) - 1))
        schedule.append(wsched)
    TOTT = t
    slotblk_core = [np.asarray(x, np.int64) for x in slotblk_core]
    for r in range(C):
        assert slotblk_core[r].size == TOTT * P, (r, slotblk_core[r].size, TOTT * P)

    idx_cores = []
    dloc_cores = []
    for r in range(C):
        msk = segkey_s // (B * NCH) == r
        key_r = segkey_s[msk] - r * nseg
        sl_r = sloc_s[msk]
        dl_r = dloc_s[msk]
        # rank of each message within its segment
        changes = np.r_[0, np.flatnonzero(np.diff(key_r)) + 1]
        seg_start_of_msg = np.repeat(changes, np.diff(np.r_[changes, key_r.size]))
        rank = np.arange(key_r.size) - seg_start_of_msg
        pos = seg_base_core[r][key_r] + rank

        SRC = np.zeros(TOTT * P, np.int16)
        DLC = np.full(TOTT * P, 255, np.int16)
        SRC[pos] = sl_r.astype(np.int16)
        DLC[pos] = dl_r.astype(np.int16)

        # idx packing for dma_gather: call-local index i -> [i%16, i//16],
        # replicated across the 8 groups of 16 partitions.  Calls are the
        # (window, chunk) segments; each is tile-aligned so packing the whole
        # stream at once keeps every call's columns self-contained.
        a = SRC.reshape(-1, 16).T                      # [16, TOTT*8]
        idx_cores.append(np.tile(a, (8, 1)).copy())    # [128, TOTT*8]
        # per-matmul dloc columns: slots of other blocks masked to 255
        DLCt = DLC.reshape(TOTT, P)
        SBt = slotblk_core[r].reshape(TOTT, P)
        cols = []
        for ws in schedule:
            for mt_l, b, _s, _e in ws["mms"]:
                mt = ws["tile0"] + mt_l
                cols.append(np.where(SBt[mt] == b, DLCt[mt], 255))
        dloc_cores.append(
            np.ascontiguousarray(np.stack(cols, axis=1)).astype(BF16)
        )                                              # [128, TOTC]

    TOTC = sum(len(ws["mms"]) for ws in schedule)
    return dict(
        dinv=dinv,
        schedule=schedule,
        TOTT=TOTT,
        TOTC=TOTC,
        cb=[int(x) for x in cb],
        idx_cores=idx_cores,
        dloc_cores=dloc_cores,
    )


def _build(pp1, pp2, with_b1, with_b2):
    """Build the 8-core SPMD bass program."""
    assert not with_b1 and not with_b2, "biases are zero in this problem"
    WTmax = max(ws["ntiles"] for p in (pp1, pp2) for ws in p["schedule"])
    WCmax = max(len(ws["mms"]) for p in (pp1, pp2) for ws in p["schedule"])

    nc = bacc.Bacc("TRN2", target_bir_lowering=False, debug=False, num_devices=C,
                   num_swdge_queues=4, dynamic_dma_scratch_size=32768)
    f32 = mybir.dt.float32
    bf16 = mybir.dt.bfloat16
    i16 = mybir.dt.int16

    t_xs = nc.dram_tensor("xs", [SP, P], bf16, kind="ExternalInput")
    t_cs = nc.dram_tensor("cs", [SP, P], bf16, kind="ExternalInput")
    t_w1 = nc.dram_tensor("w1", [P, HID], bf16, kind="ExternalInput")
    t_w2d = nc.dram_tensor("w2d", [P, P], bf16, kind="ExternalInput")
    t_dcols = nc.dram_tensor("dcols", [P, 4], bf16, kind="ExternalInput")
    t_iota = nc.dram_tensor("iota", [P, P], bf16, kind="ExternalInput")
    t_dinv = nc.dram_tensor("dinvc", [P, B], f32, kind="ExternalInput")
    t_dinv2 = nc.dram_tensor("dinv2c", [P, B], f32, kind="ExternalInput")
    t_idx1 = nc.dram_tensor("idx1", [P, pp1["TOTT"] * 8], i16,
                            kind="ExternalInput")
    t_dloc1 = nc.dram_tensor("dloc1", [P, pp1["TOTC"]], bf16,
                             kind="ExternalInput")
    t_idx2 = nc.dram_tensor("idx2", [P, pp2["TOTT"] * 8], i16,
                            kind="ExternalInput")
    t_dloc2 = nc.dram_tensor("dloc2", [P, pp2["TOTC"]], bf16,
                             kind="ExternalInput")
    t_out = nc.dram_tensor("out", [2, B, P], f32, kind="ExternalOutput")

    # separate dram tensors per table section (both the local staging shard
    # and the gathered copy): tile tracks DRAM deps per tensor, so each
    # section's collective fires as soon as its own rows are staged, and each
    # chunk's gathers wait only on their own collective
    t1_sh = nc.dram_tensor("t1sh", [SP, P], bf16, kind="Internal")
    t2_sh = nc.dram_tensor("t2sh", [SP, P], bf16, kind="Internal")
    t1_ss = [nc.dram_tensor(f"t1s{s}", [SECG[s + 1] - SECG[s], P], bf16,
                            kind="Internal", addr_space="Shared")
             for s in range(4)]
    t2_ss = [nc.dram_tensor(f"t2s{s}", [SECG[s + 1] - SECG[s], P], bf16,
                            kind="Internal", addr_space="Shared")
             for s in range(4)]

    Copy = mybir.ActivationFunctionType.Copy
    Relu = mybir.ActivationFunctionType.Relu
    Sigmoid = mybir.ActivationFunctionType.Sigmoid

    with tile.TileContext(nc) as tc:
        nc.gpsimd.load_library(mlp)
        with (
            tc.tile_pool(name="const", bufs=1) as constp,
            tc.tile_pool(name="hbuf", bufs=1) as hbufp,
            tc.tile_pool(name="io", bufs=3) as iop,
            tc.tile_pool(name="idx", bufs=5) as idxp,
            tc.tile_pool(name="msgs", bufs=2) as msgp,
            tc.tile_pool(name="oh", bufs=1) as ohp,
            tc.tile_pool(name="psA", bufs=2, space="PSUM") as psA,
            tc.tile_pool(name="psW", bufs=2, space="PSUM") as psW,
            tc.tile_pool(name="psD", bufs=1, space="PSUM") as psD,
            tc.tile_pool(name="small", bufs=4) as smallp,
        ):
            # ---- constants ----
            w1_sb = constp.tile([P, HID], bf16)
            nc.sync.dma_start(w1_sb[:], t_w1.ap())
            w2d_sb = constp.tile([P, P], bf16)
            nc.sync.dma_start(w2d_sb[:], t_w2d.ap())
            dcols_sb = constp.tile([P, 4], bf16)
            nc.sync.dma_start(dcols_sb[:], t_dcols.ap())
            iota_sb = constp.tile([P, P], bf16)
            nc.sync.dma_start(iota_sb[:], t_iota.ap())
            dinv_sb = constp.tile([P, B], f32)
            nc.sync.dma_start(dinv_sb[:], t_dinv.ap())
            dinv2_sb = constp.tile([P, B], f32)
            nc.sync.dma_start(dinv2_sb[:], t_dinv2.ap())
            ident_sb = constp.tile([P, P], f32)
            from concourse.masks import make_identity
            make_identity(nc, ident_sb[:])
            ident_bf = constp.tile([P, P], bf16)
            nc.vector.tensor_copy(ident_bf[:], ident_sb[:])

            hT_buf = hbufp.tile([P, B * P], bf16)    # layer-1 relu'd agg, [feat x node]
            own_buf = hbufp.tile([P, B * P], bf16)   # this core's table rows [node x feat]

            def ag(sh, out_t, r0, r1):
                nc.gpsimd.collective_compute(
                    "AllGather", mybir.AluOpType.bypass,
                    replica_groups=[list(range(C))],
                    ins=[sh.ap()[r0:r1, :]], outs=[out_t.ap()],
                )

            def write_sh(sh, b0, b1):
                nc.sync.dma_start(
                    sh.ap()[b0 * P:b1 * P, :]
                        .rearrange("(b p) f -> p b f", p=P),
                    own_buf[:, b0 * P:b1 * P]
                        .rearrange("p (b f) -> p b f", f=P))

            # ---- phase A: T1 = dinv * [x@W1 | c@W1]  (bf16 table) ----
            # the 4 section AllGathers launch as soon as their rows are done
            GA = 7
            for g0 in range(0, B, GA):
                ng = min(GA, B - g0)
                xt = iop.tile([P, GA * P], bf16, tag="xt")
                nc.sync.dma_start(xt[:, :ng * P],
                                  t_xs.ap()[g0 * P:(g0 + ng) * P, :],
                                  transpose=True)
                ct = iop.tile([P, GA * P], bf16, tag="ct")
                nc.sync.dma_start(ct[:, :ng * P],
                                  t_cs.ap()[g0 * P:(g0 + ng) * P, :],
                                  transpose=True)
                psg = psW.tile([P, G * P], f32, tag="psw")
                for j in range(ng):
                    nc.tensor.matmul(psg[:, j * P:j * P + HID],
                                     xt[:, j * P:(j + 1) * P],
                                     w1_sb[:], start=True, stop=True)
                    nc.tensor.matmul(psg[:, j * P + HID:(j + 1) * P],
                                     ct[:, j * P:(j + 1) * P],
                                     w1_sb[:], start=True, stop=True)
                # one batched scale per group (per-block dinv columns)
                nc.vector.tensor_tensor(
                    out=own_buf[:, g0 * P:(g0 + ng) * P]
                        .rearrange("p (b f) -> p b f", f=P),
                    in0=psg[:, :ng * P].rearrange("p (b f) -> p b f", f=P),
                    in1=dinv_sb[:, g0:g0 + ng]
                        .rearrange("p (b q) -> p b q", q=1)
                        .to_broadcast([P, ng, P]),
                    op=mybir.AluOpType.mult)
                write_sh(t1_sh, g0, g0 + ng)
                for s in range(4):
                    if (g0 + ng) * P >= SEC[s + 1] > g0 * P:
                        ag(t1_sh, t1_ss[s], SEC[s], SEC[s + 1])

            # num_idxs register per distinct size: avoids a MOVE (and its
            # WAR stall on the shared scratch register) before every gather
            nidx_regs = {}

            def nidx_reg(n):
                if n not in nidx_regs:
                    nidx_regs[n] = nc.gpsimd.to_reg(n)
                return nidx_regs[n]

            sc1_st = constp.tile([P, B], f32)
            sc2_st = constp.tile([P, B], f32)

            # ---- SpMM pass (shared for both layers) ----
            def spmm(pp, t_idx, t_dloc, tables, layer, pre, agb):
                schedule = pp["schedule"]
                cbs = pp["cb"]
                k0s = [0]
                for ws in schedule:
                    k0s.append(k0s[-1] + len(ws["mms"]))
                tiles = {}

                def load(w):
                    ws = schedule[w]
                    wt = ws["ntiles"]
                    t0 = ws["tile0"]
                    wc = len(ws["mms"])
                    idxw = idxp.tile([P, WTmax * 8], i16, tag="idxw")
                    nc.sync.dma_start(idxw[:, :wt * 8],
                                      t_idx.ap()[:, t0 * 8:(t0 + wt) * 8])
                    dlocw = idxp.tile([P, WCmax], bf16, tag="dlocw")
                    nc.sync.dma_start(dlocw[:, :wc],
                                      t_dloc.ap()[:, k0s[w]:k0s[w] + wc])
                    msgs = msgp.tile([P, WTmax * P], bf16, tag="msgs")
                    tiles[w] = (idxw, dlocw, msgs)

                def gather(w, chunks):
                    ws = schedule[w]
                    idxw, _, msgs = tiles[w]
                    # one gather per (chunk == SWDGE queue): the 4 queues run
                    # on disjoint Q7 core pairs, concurrently, and the equal
                    # chunk sizes keep them balanced
                    for c in chunks:
                        coff, cnt_t = ws["chunks"][c]
                        tbl, r0 = tables[c]
                        rows = cbs[c + 1] - cbs[c]
                        for s0 in range(0, cnt_t, 44):
                            st = min(44, cnt_t - s0)
                            o = coff + s0
                            nidx = st * P
                            nc.gpsimd.dma_gather(
                                msgs[:, o * P:(o + st) * P].rearrange(
                                    "p (t d) -> p t d", d=P),
                                tbl.ap()[r0:r0 + rows, :],
                                idxw[:, o * 8:(o + st) * 8],
                                nidx, nidx_reg(nidx), P, single_packet=False,
                                queue_num=(c + w) % 4,
                            )

                def compute(w):
                    ws = schedule[w]
                    wc = len(ws["mms"])
                    _, dlocw, msgs = tiles[w]
                    ohg = ohp.tile([P, WCmax * P], bf16, tag="ohg")
                    # build the one-hots in pieces so the first matmuls can
                    # start while DVE still builds the tail
                    ohc = -(-wc // 3)
                    for j0 in range(0, wc, ohc):
                        jn = min(ohc, wc - j0)
                        nc.vector.tensor_tensor(
                            out=ohg[:, j0 * P:(j0 + jn) * P]
                                .rearrange("p (t d) -> p t d", d=P),
                            in0=dlocw[:, j0:j0 + jn].to_broadcast([P, jn, P]),
                            in1=iota_sb[:].rearrange("p (a d) -> p a d", a=1)
                                .to_broadcast([P, jn, P]),
                            op=mybir.AluOpType.is_equal)
                    # transposed aggregation: psw[feat, dst] += msgs^T @ onehot
                    psw = psW.tile([P, G * P], f32, tag="psw")
                    for k, (mt_l, b, st_f, sp_f) in enumerate(ws["mms"]):
                        bw = b - w * G
                        if st_f:
                            # self-loop: psum[:, d] += own_buf[d, :]^T
                            nc.tensor.matmul(
                                psw[:, bw * P:(bw + 1) * P],
                                own_buf[:, b * P:(b + 1) * P], ident_bf[:],
                                start=True, stop=False)
                        nc.tensor.matmul(
                            psw[:, bw * P:(bw + 1) * P],
                            msgs[:, mt_l * P:(mt_l + 1) * P],
                            ohg[:, k * P:(k + 1) * P],
                            start=False, stop=sp_f)
                    if layer == 1:
                        for bw in range(G):
                            gb = w * G + bw
                            # h~ = relu(agg); dst dinv deferred (relu commutes
                            # with the positive diagonal scale)
                            nc.scalar.activation(
                                hT_buf[:, gb * P:(gb + 1) * P],
                                psw[:, bw * P:(bw + 1) * P], Relu)
                            # phase C pipelined: T2 = dinv^2 * (h~ @ W2d)
                            ps = psA.tile([P, P], f32, tag="psd")
                            nc.tensor.matmul(ps[:],
                                             hT_buf[:, gb * P:(gb + 1) * P],
                                             w2d_sb[:], start=True, stop=True)
                            nc.scalar.activation(own_buf[:, gb * P:(gb + 1) * P],
                                                 ps[:], Copy,
                                                 scale=dinv2_sb[:, gb:gb + 1])
                        write_sh(t2_sh, w * G, (w + 1) * G)
                        # T2 section collectives launch mid-stream, masked
                        # by the in-flight gathers
                        for s in range(3):
                            if (w + 1) * G * P >= SEC[s + 1] > w * G * P:
                                ag(t2_sh, t2_ss[s], SEC[s], SEC[s + 1])
                    else:
                        # discriminator, inline: per block one [128x4] matmul
                        # off the (unscaled) aggregation copy
                        y2w = smallp.tile([P, G * P], bf16, tag="y2w")
                        psd = psD.tile([P, 4 * G], f32, tag="psd2")
                        for bw in range(G):
                            nc.scalar.activation(
                                y2w[:, bw * P:(bw + 1) * P],
                                psw[:, bw * P:(bw + 1) * P], Copy)
                            nc.tensor.matmul(
                                psd[:, bw * 4:(bw + 1) * 4],
                                y2w[:, bw * P:(bw + 1) * P], dcols_sb[:],
                                start=True, stop=True)
                        # scale all rows by dst dinv, then
                        # sc_j = (w . y2) * sigmoid(dinv * mean(h1-part))
                        sd = smallp.tile([P, 4 * G], f32, tag="sd")
                        sdv = sd[:].rearrange("p (b q) -> p b q", q=4)
                        nc.vector.tensor_tensor(
                            out=sdv,
                            in0=psd[:].rearrange("p (b q) -> p b q", q=4),
                            in1=dinv_sb[:, w * G:(w + 1) * G]
                                .rearrange("p (b q) -> p b q", q=1)
                                .to_broadcast([P, G, 4]),
                            op=mybir.AluOpType.mult)
                        ccol = smallp.tile([P, G], f32, tag="ccol")
                        ccol3 = ccol[:].rearrange("p (b q) -> p b q", q=1)
                        nc.scalar.activation(ccol3, sdv[:, :, 0:1], Sigmoid)
                        nc.vector.tensor_tensor(
                            out=sc1_st[:, w * G:(w + 1) * G]
                                .rearrange("p (b q) -> p b q", q=1),
                            in0=sdv[:, :, 1:2],
                            in1=ccol3, op=mybir.AluOpType.mult)
                        nc.vector.tensor_tensor(
                            out=sc2_st[:, w * G:(w + 1) * G]
                                .rearrange("p (b q) -> p b q", q=1),
                            in0=sdv[:, :, 2:3],
                            in1=ccol3, op=mybir.AluOpType.mult)
                        # output per table half: transpose sc -> [blk, P]
                        # and store, so the tail only drains the last half
                        if w in (6, NW - 1):
                            b0, b1 = (0, 49) if w == 6 else (49, B)
                            for j, st in enumerate((sc1_st, sc2_st)):
                                pso = psD.tile([49, P], f32, tag="pstr")
                                nc.tensor.transpose(pso[:], st[:, b0:b1],
                                                    ident_sb[:])
                                so = smallp.tile([49, P], f32, tag="so")
                                nc.scalar.activation(so[:], pso[:], Copy)
                                nc.sync.dma_start(t_out.ap()[j][b0:b1, :],
                                                  so[:])

                # prelude: the first two windows' already-gatherable chunks
                # are emitted BEFORE the last collective, so the Pool engine
                # streams useful gathers while the collective's data-wait +
                # CC execution run
                rest = [c for c in range(4) if c not in pre]
                load(0)
                gather(0, pre)
                load(1)
                gather(1, pre)
                agb()
                gather(0, rest)
                gather(1, rest)
                compute(0)
                compute(1)
                for w in range(2, NW):
                    load(w)
                    gather(w, [0, 1, 2, 3])
                    compute(w)

            spmm(pp1, t_idx1, t_dloc1,
                 [(t1_ss[0], 0), (t1_ss[1], 0), (t1_ss[2], 0), (t1_ss[3], 0)],
                 layer=1, pre=[0, 1, 2, 3], agb=lambda: None)
            spmm(pp2, t_idx2, t_dloc2,
                 [(t2_ss[0], 0), (t2_ss[1], 0), (t2_ss[2], 0), (t2_ss[3], 0)],
                 layer=2, pre=[0, 1, 2],
                 agb=lambda: ag(t2_sh, t2_ss[3], SEC[3], SP))

    nc.compile()
    return nc


_CACHE = {}


def _run(inputs, trace=False):
    x = np.asarray(inputs["x"], np.float32)
    cfeat = np.asarray(inputs["cfeat"], np.float32)
    edge_index = inputs["edge_index"]
    W1 = np.asarray(inputs["W1"], np.float32)
    b1 = np.asarray(inputs["b1"], np.float32)
    W2 = np.asarray(inputs["W2"], np.float32)
    b2 = np.asarray(inputs["b2"], np.float32)
    Wd = np.asarray(inputs["Wd"], np.float32)
    bd = np.asarray(inputs["bd"], np.float32)

    pp1 = _preprocess(edge_index, "sec4")
    pp2 = pp1
    with_b1 = bool(np.any(b1 != 0))
    with_b2 = bool(np.any(b2 != 0))

    key = ("nc", with_b1, with_b2, int(np.asarray(edge_index)[0, 0]),
           pp1["TOTT"], pp2["TOTT"])
    if key not in _CACHE:
        _CACHE.clear()
        _CACHE[key] = _build(pp1, pp2, with_b1, with_b2)
    nc = _CACHE[key]

    w = Wd.sum(axis=1).astype(np.float32)          # [64]
    # disc matmul columns: [mean-input(h1) | w.h1 | w.h2 | 0]
    dcols = np.zeros((P, 4), np.float32)
    dcols[:HID, 0] = 1.0 / HID
    dcols[:HID, 1] = w
    dcols[HID:, 2] = w
    dcols = dcols.astype(BF16)
    w1b = W1.astype(BF16)
    w2d = np.zeros((P, P), np.float32)
    w2d[:HID, :HID] = W2
    w2d[HID:, HID:] = W2
    w2d = w2d.astype(BF16)
    iota = np.tile(np.arange(P, dtype=np.float32)[None, :], (P, 1)).astype(BF16)

    dinv = pp1["dinv"]
    in_maps = []
    for r in range(C):
        xs = np.zeros((SP, P), np.float32)
        xs[:S] = x[r * S:(r + 1) * S]
        cs = np.zeros((SP, P), np.float32)
        cs[:S] = cfeat[r * S:(r + 1) * S]
        dv = np.ones(SP, np.float32)
        dv[:S] = dinv[r * S:(r + 1) * S]
        dvc = np.ascontiguousarray(dv.reshape(B, P).T)
        in_maps.append(dict(
            xs=xs.astype(BF16), cs=cs.astype(BF16),
            w1=w1b, w2d=w2d, dcols=dcols, iota=iota,
            dinvc=dvc, dinv2c=dvc * dvc,
            idx1=pp1["idx_cores"][r], dloc1=pp1["dloc_cores"][r],
            idx2=pp2["idx_cores"][r], dloc2=pp2["dloc_cores"][r],
        ))

    res = bass_utils.run_bass_kernel_spmd(
        nc, in_maps, core_ids=list(range(C)), trace=trace)

    sc1 = np.empty(N, np.float32)
    sc2 = np.empty(N, np.float32)
    for r in range(C):
        o = res.results[r]["out"].reshape(2, SP)
        sc1[r * S:(r + 1) * S] = o[0, :S]
        sc2[r * S:(r + 1) * S] = o[1, :S]
    logits = np.concatenate([sc1 + bd[0], sc2 + bd[0]])[None, :].astype(np.float32)
    return logits, res


def kernel(**inputs):
    logits, _ = _run(inputs, trace=False)
    return logits
